# revision 15
# baseline (speedup 1.0000x reference)
"""DiffAE attention block (GroupNorm -> qkv 1x1conv -> attention -> proj -> residual)
as a Bass/Tile kernel on 8 TRN2 NeuronCores.

Sharding: data-parallel over batch. B=32 samples, 4 per core. Attention is
per-sample, so no collectives are needed: inputs are sharded host-side and
outputs gathered host-side.

Math restructure vs the straightforward reference:
  * proj is folded into the v weights host-side: W' = proj_w @ wv; since the
    per-column softmax scale commutes with the channel projection and softmax
    rows sum to exactly 1 against the kernel's own denominator, the v/proj
    biases collapse to a constant output bias bo = proj_w @ bv + pb. This
    removes the whole proj matmul stage (12.5% of the FLOPs). bo itself is
    injected through a rank-1 fp8 matmul against the (rescaled) softmax
    denominator, appended to each attn@v accumulation group, so the final
    epilogue is out = h2*rs + x, with no extra elementwise pass.
  * All four remaining matmul stages (q, k, scores, attn@v) run in fp8-e4m3
    with perf_mode=DoubleRow (K=256/instruction, ~216ns for a
    [K=256]x[128,2x512] instruction = ~1.9x bf16 FLOP rate), fp32 PSUM.
  * fp8 scaling: TRN e4m3 spans [2^-9, 240]. GroupNorm output h is stored
    x8, q/k/v weights x4 (q,k,v tiles 32x true). Scores PSUM = 1024x true;
    exp applies scale SCALE/1024 and bias -2 so max e ~40 << 240 (softmax
    shift-invariance cancels the -2). The denominator matmul constant is
    32.0 so rs = 1/(32 sum e) cancels the 32x in v; the denominator is also
    evicted as fp8 at x2^-8 for the bo bias matmul (bo lhsT carries x256).

Engine assignment and emission order are tuned against the PE's in-order
queue (trace-driven): PSUM tiles are double-bank [128, 2, 512] so each
eviction/exp is one wide op; scores chunk 1 is emitted before attn@v chunk 0
so the PE has ready work while exp drains; sample s+1's GroupNorm work is
split so its tiny PE matmuls (bf16, fast path) sit in the PE queue only
where their DVE inputs are already done.
  ACT : exp, v evict (Copy), denominator fp8 evict, GroupNorm affine
        (exp is the only table function -> single table load ever)
  DVE : q/k evict+bias, bn_stats, Newton rsqrt (no ACT Sqrt), reciprocal,
        h2*rs
  Pool: final out = t + x (plain SBUF add)
  PE  : all matmuls incl. softmax denominator (32.0-matmul DoubleRow)
"""

import numpy as np
import ml_dtypes

import concourse.bacc as bacc
import concourse.bass as bass
import concourse.mybir as mybir
import concourse.tile as tile
from concourse import bass_isa
from concourse.bass_utils import run_bass_kernel_spmd

N_CORES = 8
B, C, H, W = 32, 512, 32, 32
HW = H * W                      # 1024 spatial positions
BS = B // N_CORES               # 4 samples per core
GROUPS = 32
EPS = 1e-5
SCALE = float(C) ** -0.5
P = 128
CT = C // P                     # 4 channel tiles
MT = HW // P                    # 8 spatial tiles
KP = CT // 2                    # 2 DoubleRow contraction pairs over channels
MP = MT // 2                    # 4 DoubleRow contraction pairs over spatial
NF = 512                        # matmul moving-dim chunk (output columns)
NCH = HW // NF                  # 2 column chunks
F32 = mybir.dt.float32
I32 = mybir.dt.int32
BF16 = mybir.dt.bfloat16
F8 = mybir.dt.float8e4
AX = mybir.AxisListType
ALU = mybir.AluOpType
ACTF = mybir.ActivationFunctionType
DR = mybir.MatmulPerfMode.DoubleRow

H_SC = 8.0                      # h stored as 8*h
W_SC = 4.0                      # q/k/v weights stored as 4*W
QK_SC = H_SC * W_SC             # q,k tiles are 32x true
S_SC = QK_SC * QK_SC            # scores PSUM is 1024x true
E_BIAS = -2.0                   # exp(s - 2): keeps max e ~40 << 240 (fp8 max)
ONE_V = 32.0                    # denominator matmul constant; 1/(32 sum e)
                                # cancels the 32x in the v tiles
DN_SC = 1.0 / 256.0             # denominator fp8 evict scale (keeps ~35 max)
BO_SC = 256.0                   # bo lhsT scale: bo9 @ dn8 = bo * 32 sum e
RSQRT_MAGIC = 0x5F3759DF
RSQRT_SUB = 0x7FFFFFFF - RSQRT_MAGIC


def build():
    nc = bacc.Bacc("TRN2", target_bir_lowering=False, debug=False,
                   num_devices=N_CORES, num_swdge_queues=4)

    x_d = nc.declare_dram_parameter("x", [BS, C, HW], F32, isOutput=False)
    wq_d = nc.declare_dram_parameter("wq", [P, KP, 2, C], F8, isOutput=False)
    wk_d = nc.declare_dram_parameter("wk", [P, KP, 2, C], F8, isOutput=False)
    wv_d = nc.declare_dram_parameter("wv", [P, KP, 2, C], F8, isOutput=False)
    bo9_d = nc.declare_dram_parameter("bo9", [P, C], F8, isOutput=False)
    gm_d = nc.declare_dram_parameter("gm", [P, CT, GROUPS], BF16, isOutput=False)
    gmT_d = nc.declare_dram_parameter("gmT", [GROUPS, C], BF16, isOutput=False)
    bq_d = nc.declare_dram_parameter("bq", [P, CT], F32, isOutput=False)
    bk_d = nc.declare_dram_parameter("bk", [P, CT], F32, isOutput=False)
    gnw_d = nc.declare_dram_parameter("gnw", [P, CT], F32, isOutput=False)
    gnb_d = nc.declare_dram_parameter("gnb", [P, CT], F32, isOutput=False)
    out_d = nc.declare_dram_parameter("out", [BS, C, HW], F32, isOutput=True)

    with tile.TileContext(nc) as tc:
        build_tile(tc, x_d, wq_d, wk_d, wv_d, bo9_d, gm_d, gmT_d,
                   bq_d, bk_d, gnw_d, gnb_d, out_d)
    nc.finalize()
    return nc


def build_tile(tc, x_d, wq_d, wk_d, wv_d, bo9_d, gm_d, gmT_d,
               bq_d, bk_d, gnw_d, gnb_d, out_d):
    nc = tc.nc
    from contextlib import ExitStack
    with ExitStack() as ctx:
        ctx.enter_context(nc.allow_low_precision(
            reason="fp8 DoubleRow matmuls; fp32 accumulate in PSUM"))
        consts = ctx.enter_context(tc.tile_pool(name="consts", bufs=1))
        xs = ctx.enter_context(tc.tile_pool(name="xs", bufs=8))
        hp = ctx.enter_context(tc.tile_pool(name="hp", bufs=2))
        qp = ctx.enter_context(tc.tile_pool(name="qp", bufs=2))
        kp_ = ctx.enter_context(tc.tile_pool(name="kp", bufs=2))
        vp = ctx.enter_context(tc.tile_pool(name="vp", bufs=2))
        ep = ctx.enter_context(tc.tile_pool(name="ep", bufs=4))
        rp = ctx.enter_context(tc.tile_pool(name="rp", bufs=3))
        dp = ctx.enter_context(tc.tile_pool(name="dp", bufs=3))
        op = ctx.enter_context(tc.tile_pool(name="op", bufs=10))
        st = ctx.enter_context(tc.tile_pool(name="st", bufs=16))
        pgn = ctx.enter_context(tc.tile_pool(name="pgn", bufs=1, space="PSUM"))
        pm1 = ctx.enter_context(tc.tile_pool(name="pm1", bufs=1, space="PSUM"))
        pm2 = ctx.enter_context(tc.tile_pool(name="pm2", bufs=3, space="PSUM"))

        # ---- constants / weights ----
        gm_sb = consts.tile([P, CT, GROUPS], BF16, name="gm_sb")
        gmT_sb = consts.tile([GROUPS, C], BF16, name="gmT_sb")
        bq_sb = consts.tile([P, CT], F32, name="bq_sb")
        bk_sb = consts.tile([P, CT], F32, name="bk_sb")
        gnw_sb = consts.tile([P, CT], F32, name="gnw_sb")
        gnb_sb = consts.tile([P, CT], F32, name="gnb_sb")
        epsg_sb = consts.tile([GROUPS, 1], F32, name="epsg_sb")
        ebias_sb = consts.tile([P, 1], F32, name="ebias_sb")
        ones_sb = consts.tile([P, 2, P], F8, name="ones_sb")
        bo9_sb = consts.tile([P, C], F8, name="bo9_sb")
        wq_sb = consts.tile([P, KP, 2, C], F8, name="wq_sb")
        wk_sb = consts.tile([P, KP, 2, C], F8, name="wk_sb")
        wv_sb = consts.tile([P, KP, 2, C], F8, name="wv_sb")

        for sb, d in ((gm_sb, gm_d), (gmT_sb, gmT_d), (bo9_sb, bo9_d),
                      (bq_sb, bq_d), (bk_sb, bk_d),
                      (gnw_sb, gnw_d), (gnb_sb, gnb_d)):
            nc.gpsimd.dma_start(out=sb, in_=d[:])
        nc.vector.memset(epsg_sb, EPS)
        nc.vector.memset(ebias_sb, E_BIAS)
        nc.vector.memset(ones_sb, ONE_V)

        inv_gsz = 1.0 / (C // GROUPS * HW)

        def prep_load(s):
            """x DMA for sample s."""
            x_t = []
            for ct in range(CT):
                xt = xs.tile([P, HW], F32, name=f"x_s{s}_{ct}", tag="x")
                for n in range(NCH):
                    nsl = slice(n * NF, (n + 1) * NF)
                    nc.sync.dma_start(out=xt[:, nsl],
                                      in_=x_d[s, ct * P:(ct + 1) * P, nsl])
                x_t.append(xt)
            if s == 0:
                for sb, d in ((wq_sb, wq_d), (wk_sb, wk_d), (wv_sb, wv_d)):
                    nc.gpsimd.dma_start(out=sb, in_=d[:])
            return x_t

        def stats_a(s, x_t):
            """bn_stats row stats + group-sum matmuls -> gsum (PSUM).
            gm carries x1024 so me is just [mean, E[x^2]] per row (bf16)."""
            gsum = pgn.tile([GROUPS, 2], F32, name=f"gsum_{s}", tag="pg")
            bnsts = []
            for ct in range(CT):
                bnst = st.tile([P, NCH, 6], F32, name=f"bnst_{s}_{ct}",
                               tag=f"bnst{ct}")
                xv = x_t[ct].rearrange("p (a b) -> p a b", b=NF)
                for sg in range(NCH):
                    nc.vector.bn_stats(bnst[:, sg, :], xv[:, sg, :])
                bnsts.append(bnst)
            for ct in range(CT):
                rowmv = st.tile([P, 2], F32, name=f"rowmv_{s}_{ct}",
                                tag="rowmv")
                nc.vector.bn_aggr(rowmv, bnsts[ct])
                me = st.tile([P, 2], BF16, name=f"me_{s}_{ct}", tag="me")
                nc.vector.tensor_copy(me[:, 0:1], rowmv[:, 0:1])
                nc.vector.scalar_tensor_tensor(
                    out=me[:, 1:2], in0=rowmv[:, 0:1],
                    scalar=rowmv[:, 0:1], in1=rowmv[:, 1:2],
                    op0=ALU.mult, op1=ALU.add)
                nc.tensor.matmul(gsum, lhsT=gm_sb[:, ct, :], rhs=me,
                                 start=(ct == 0), stop=(ct == CT - 1))
            return gsum

        def stats_rsqrt(s, gsum):
            """mean + Newton-rsqrt(var+eps) -> mv (bf16, for expand MM)."""
            mv = st.tile([GROUPS, 2], BF16, name=f"mv_{s}", tag="mv")
            nc.scalar.mul(out=mv[:, 0:1], in_=gsum[:, 0:1], mul=inv_gsz)
            ex2 = st.tile([GROUPS, 1], F32, name=f"ex2_{s}", tag="ex2")
            nc.scalar.mul(out=ex2, in_=gsum[:, 1:2], mul=inv_gsz)
            msq = st.tile([GROUPS, 1], F32, name=f"msq_{s}", tag="msq")
            nc.vector.tensor_mul(msq, mv[:, 0:1], mv[:, 0:1])
            vpe = st.tile([GROUPS, 1], F32, name=f"vpe_{s}", tag="vpe")
            nc.vector.tensor_sub(vpe, ex2, msq)
            nc.vector.tensor_add(vpe, vpe, epsg_sb)
            y = st.tile([GROUPS, 1], F32, name=f"y_{s}", tag="y")
            yi = y.bitcast(I32)
            vi = vpe.bitcast(I32)
            nc.vector.tensor_scalar(out=yi, in0=vi, scalar1=1,
                                    scalar2=0x7FFFFFFF,
                                    op0=ALU.logical_shift_right,
                                    op1=ALU.bitwise_xor)
            nc.vector.tensor_scalar(out=yi, in0=yi, scalar1=RSQRT_SUB,
                                    scalar2=None, op0=ALU.subtract)
            a = st.tile([GROUPS, 1], F32, name=f"a_{s}", tag="a")
            for it in range(2):
                nc.vector.tensor_mul(a, y, y)
                nc.vector.tensor_mul(a, a, vpe)
                nc.vector.tensor_scalar(out=a, in0=a, scalar1=-0.5,
                                        scalar2=1.5, op0=ALU.mult,
                                        op1=ALU.add)
                dst = mv[:, 1:2] if it == 1 else y
                nc.vector.tensor_mul(dst, y, a)
            return mv

        def stats_expand(s, mv):
            """expand (mean, rstd) to per-channel alpha/beta in one batch of
            [128, CT]-wide ops (tiny bf16 matmuls + 4 DVE ops)."""
            eps4 = pgn.tile([P, CT, 2], F32, name=f"eps4_{s}", tag="pg")
            for ct in range(CT):
                nc.tensor.matmul(eps4[:, ct, :],
                                 lhsT=gmT_sb[:, ct * P:(ct + 1) * P],
                                 rhs=mv, start=(ct == 0), stop=(ct == CT - 1),
                                 skip_group_check=True)
            exs = st.tile([P, CT, 2], F32, name=f"exs_{s}", tag="exs")
            nc.vector.tensor_copy(exs, eps4)
            alpha = st.tile([P, CT], F32, name=f"al_{s}", tag="al")
            nc.vector.tensor_mul(alpha, gnw_sb, exs[:, :, 1])
            mal = st.tile([P, CT], F32, name=f"mal_{s}", tag="mal")
            nc.vector.tensor_mul(mal, exs[:, :, 0], alpha)
            beta = st.tile([P, CT], F32, name=f"be_{s}", tag="be")
            nc.vector.tensor_sub(beta, gnb_sb, mal)
            return alpha, beta

        def stats_affine(s, x_t, alpha, beta):
            """ACT affine -> h (fp8, 8x scaled)."""
            ht = hp.tile([P, CT, HW], F8, name=f"h_{s}", tag="h")
            for ct in range(CT):
                nc.scalar.activation(out=ht[:, ct, :], in_=x_t[ct],
                                     func=ACTF.Identity,
                                     bias=beta[:, ct:ct + 1],
                                     scale=alpha[:, ct:ct + 1])
            return ht

        def body_qkv(s, h_t):
            """q, k (channel-major) and vT (spatial-major) for sample s."""
            q_t = qp.tile([P, CT, HW], F8, name=f"q_{s}", tag="q")
            k_t = kp_.tile([P, CT, HW], F8, name=f"k_{s}", tag="k")
            for w_sb, b_sb, dst, tag in ((wq_sb, bq_sb, q_t, "q"),
                                         (wk_sb, bk_sb, k_t, "k")):
                for mt in range(CT):
                    ps2 = pm2.tile([P, NCH, NF], F32,
                                   name=f"{tag}p_{s}_{mt}", tag="ps")
                    for kpi in range(KP):
                        lhsT = w_sb[:, kpi, :, mt * P:(mt + 1) * P]
                        for n in range(NCH):
                            nsl = slice(n * NF, (n + 1) * NF)
                            nc.tensor.matmul(
                                ps2[:, n, :], lhsT=lhsT,
                                rhs=h_t[:, 2 * kpi:2 * kpi + 2, nsl],
                                start=(kpi == 0), stop=(kpi == KP - 1),
                                perf_mode=DR)
                    nc.vector.tensor_scalar(
                        out=dst[:, mt, :],
                        in0=ps2.rearrange("p a b -> p (a b)"),
                        scalar1=b_sb[:, mt:mt + 1], scalar2=None,
                        op0=ALU.add)

            vT = vp.tile([P, MT, C], F8, name=f"v_{s}", tag="v")
            for mtp in range(MT // 2):
                ps2 = pm2.tile([P, 2, C], F32, name=f"vp_{s}_{mtp}", tag="ps")
                for i in range(2):
                    mt = 2 * mtp + i
                    for kpi in range(KP):
                        nc.tensor.matmul(
                            ps2[:, i, :],
                            lhsT=h_t[:, 2 * kpi:2 * kpi + 2,
                                     mt * P:(mt + 1) * P],
                            rhs=wv_sb[:, kpi, :, :],
                            start=(kpi == 0), stop=(kpi == KP - 1),
                            perf_mode=DR)
                nc.scalar.activation(out=vT[:, 2 * mtp:2 * mtp + 2, :],
                                     in_=ps2, func=ACTF.Copy)
            return q_t, k_t, vT

        def attn_scores(s, n, q_t, k_t):
            """scores + exp for column chunk n -> e super-tile."""
            nsl = slice(n * NF, (n + 1) * NF)
            e_t = ep.tile([P, MT, NF], F8, name=f"e_{s}_{n}", tag="e")
            for mtp in range(MT // 2):
                ps2 = pm2.tile([P, 2, NF], F32, name=f"ep_{s}_{n}_{mtp}",
                               tag="ps")
                for i in range(2):
                    mt = 2 * mtp + i
                    for kpi in range(KP):
                        nc.tensor.matmul(
                            ps2[:, i, :],
                            lhsT=k_t[:, 2 * kpi:2 * kpi + 2,
                                     mt * P:(mt + 1) * P],
                            rhs=q_t[:, 2 * kpi:2 * kpi + 2, nsl],
                            start=(kpi == 0), stop=(kpi == KP - 1),
                            perf_mode=DR)
                nc.scalar.activation(out=e_t[:, 2 * mtp:2 * mtp + 2, :],
                                     in_=ps2, func=ACTF.Exp,
                                     scale=SCALE / S_SC, bias=ebias_sb)
            return e_t

        def attn_out(s, n, x_t, vT, e_t, last=False):
            """denominator + (v' @ e^T ... + bo-bias matmul) * rs + x,
            interleaved as mp-waves so the PE starts on the first exp
            pairs immediately."""
            nsl = slice(n * NF, (n + 1) * NF)
            den = pm1.tile([P, NF], F32, name=f"dn_{s}_{n}", tag="pd")
            h2 = [pm2.tile([P, 2, NF], F32, name=f"h2_{s}_{n}_{cp}", tag="ps")
                  for cp in range(CT // 2)]
            for mp in range(MP):
                epair = e_t[:, 2 * mp:2 * mp + 2, :]
                nc.tensor.matmul(den, lhsT=ones_sb, rhs=epair,
                                 start=(mp == 0), stop=(mp == MP - 1),
                                 perf_mode=DR)
                for cp in range(CT // 2):
                    for i in range(2):
                        ct = 2 * cp + i
                        nc.tensor.matmul(
                            h2[cp][:, i, :],
                            lhsT=vT[:, 2 * mp:2 * mp + 2,
                                    ct * P:(ct + 1) * P],
                            rhs=epair,
                            start=(mp == 0), stop=False,
                            perf_mode=DR)
            # denominator: fp8 evict (for the bo bias matmul) + reciprocal
            dn8 = dp.tile([P, NF], F8, name=f"dn8_{s}_{n}", tag="dn8")
            nc.scalar.activation(out=dn8, in_=den, func=ACTF.Copy,
                                 scale=DN_SC)
            rs = rp.tile([P, NF], F32, name=f"rs_{s}_{n}", tag="rs")
            nc.vector.reciprocal_approx_fast(out=rs, in_=den)
            # close each h2 group with the rank-1 bo matmul:
            # h2 += bo9[:, ct-slice].T @ dn8 = 256*bo * (32 sum e)/256
            for cp in range(CT // 2):
                for i in range(2):
                    ct = 2 * cp + i
                    nc.tensor.matmul(
                        h2[cp][:, i, :],
                        lhsT=bo9_sb[:, ct * P:(ct + 1) * P],
                        rhs=dn8, start=False, stop=True,
                        skip_group_check=True)
            for cp in range(CT // 2):
                for i in range(2):
                    ct = 2 * cp + i
                    t_sb = op.tile([P, NF], F32, name=f"t_{s}_{n}_{ct}",
                                   tag="t")
                    nc.vector.tensor_mul(t_sb, h2[cp][:, i, :], rs)
                    o_sb = op.tile([P, NF], F32, name=f"o_{s}_{n}_{ct}",
                                   tag="o")
                    # last chunk: split the serial drain across DVE and Pool
                    if last and ct >= 2:
                        nc.vector.tensor_add(o_sb, t_sb, x_t[ct][:, nsl])
                    else:
                        nc.gpsimd.tensor_add(o_sb, t_sb, x_t[ct][:, nsl])
                    nc.sync.dma_start(
                        out=out_d[s, ct * P:(ct + 1) * P, nsl], in_=o_sb)

        # ---- software pipeline over samples ----
        x0 = prep_load(0)
        g0 = stats_a(0, x0)
        mv0 = stats_rsqrt(0, g0)
        a0, b0 = stats_expand(0, mv0)
        h0 = stats_affine(0, x0, a0, b0)
        cur = (x0, h0)
        for s in range(BS):
            x_t, h_t = cur
            nxt_x = prep_load(s + 1) if s + 1 < BS else None
            q_t, k_t, vT = body_qkv(s, h_t)
            e0 = attn_scores(s, 0, q_t, k_t)
            # scores chunk 1 before attn chunk 0: the PE has ready work
            # while ACT drains exp of chunk 0
            e1 = attn_scores(s, 1, q_t, k_t)
            # s+1 GroupNorm stats + rsqrt before any epilogue DVE work so
            # the expand matmuls/affine never gate on the attn drain
            if nxt_x is not None:
                gsum = stats_a(s + 1, nxt_x)
                mvn = stats_rsqrt(s + 1, gsum)
            attn_out(s, 0, x_t, vT, e0)
            ab = stats_expand(s + 1, mvn) if nxt_x is not None else None
            attn_out(s, 1, x_t, vT, e1, last=(s == BS - 1))
            nxt = ((nxt_x, stats_affine(s + 1, nxt_x, *ab))
                   if nxt_x is not None else None)
            cur = nxt


_NC_CACHE = None


def _get_nc():
    global _NC_CACHE
    if _NC_CACHE is None:
        _NC_CACHE = build()
    return _NC_CACHE


F8NP = ml_dtypes.float8_e4m3


def _tile_w_dr(w):
    """[512 out, 512 in] weight -> DoubleRow lhsT tiles [P, KP, 2, C]:
    [p, kp, i, o] = w[o, (kp*2 + i)*128 + p], scaled by W_SC, fp8."""
    wT = (W_SC * w.T).astype(np.float32)          # [c_in, o]
    return np.ascontiguousarray(
        wT.reshape(KP, 2, P, C).transpose(2, 0, 1, 3)).astype(F8NP)


def _tile_vec(v, scale=1.0):
    """[512] -> [128, 4] per-partition scalars: [p, kt] = scale*v[kt*128+p]"""
    return np.ascontiguousarray(
        (scale * np.asarray(v, dtype=np.float64)).astype(np.float32)
        .reshape(CT, P).T)


def make_in_maps(x, gn_w, gn_b, qkv_w, qkv_b, proj_w, proj_b):
    x = np.asarray(x, dtype=np.float32)
    gn_w = np.asarray(gn_w, dtype=np.float32)
    gn_b = np.asarray(gn_b, dtype=np.float32)
    qkv_w = np.asarray(qkv_w, dtype=np.float32)
    qkv_b = np.asarray(qkv_b, dtype=np.float32)
    proj_w = np.asarray(proj_w, dtype=np.float32)
    proj_b = np.asarray(proj_b, dtype=np.float32)

    xr = x.reshape(B, C, HW)
    gmat = np.kron(np.eye(GROUPS, dtype=np.float32),
                   np.ones((C // GROUPS, 1), dtype=np.float32))  # [512, 32]
    gm_t = (float(HW) * gm_t_base(gmat)).astype(ml_dtypes.bfloat16)
    gmT_t = np.ascontiguousarray(gmat.T).astype(ml_dtypes.bfloat16)

    # fold proj into v: W' = proj_w @ wv; bias collapses to a constant
    # output offset bo = proj_w @ bv + pb (softmax rows sum to 1)
    wv_folded = proj_w @ qkv_w[2 * C:3 * C]
    bo = proj_w @ qkv_b[2 * C:3 * C] + proj_b
    bo9 = np.zeros((P, C), dtype=np.float32)
    bo9[0, :] = BO_SC * bo

    common = {
        "wq": _tile_w_dr(qkv_w[0:C]),
        "wk": _tile_w_dr(qkv_w[C:2 * C]),
        "wv": _tile_w_dr(wv_folded),
        "bo9": bo9.astype(F8NP),
        "gm": gm_t,
        "gmT": gmT_t,
        "bq": _tile_vec(qkv_b[0:C], QK_SC),
        "bk": _tile_vec(qkv_b[C:2 * C], QK_SC),
        "gnw": _tile_vec(gn_w, H_SC),
        "gnb": _tile_vec(gn_b, H_SC),
    }
    in_maps = []
    for c in range(N_CORES):
        m = dict(common)
        m["x"] = np.ascontiguousarray(xr[c * BS:(c + 1) * BS])
        in_maps.append(m)
    return in_maps


def gm_t_base(gmat):
    return np.ascontiguousarray(
        gmat.reshape(CT, P, GROUPS).transpose(1, 0, 2)).astype(np.float32)


def kernel(**inputs):
    in_maps = make_in_maps(**inputs)
    nc = _get_nc()
    res = run_bass_kernel_spmd(nc, in_maps, core_ids=list(range(N_CORES)))
    out = np.concatenate([res.results[c]["out"] for c in range(N_CORES)],
                         axis=0)
    return out.reshape(B, C, H, W).astype(np.float32)


# revision 16
# speedup vs baseline: 1.2623x; 1.2623x over previous
"""DiffAE attention block (GroupNorm -> qkv 1x1conv -> attention -> proj -> residual)
as a Bass/Tile kernel on 8 TRN2 NeuronCores.

Sharding: data-parallel over batch. B=32 samples, 4 per core. Attention is
per-sample, so no collectives are needed: inputs are sharded host-side and
outputs gathered host-side.

Math restructure vs the straightforward reference:
  * proj is folded into the v weights host-side: W' = proj_w @ wv; since the
    per-column softmax scale commutes with the channel projection and softmax
    rows sum to exactly 1 against the kernel's own denominator, the v/proj
    biases collapse to a constant output bias bo = proj_w @ bv + pb. This
    removes the whole proj matmul stage (12.5% of the FLOPs). bo itself is
    injected through a rank-1 fp8 matmul against the (rescaled) softmax
    denominator, appended to each attn@v accumulation group, so the final
    epilogue is out = h2*rs + x, with no extra elementwise pass.
  * All four remaining matmul stages (q, k, scores, attn@v) run in fp8-e4m3
    with perf_mode=DoubleRow (K=256/instruction, ~216ns for a
    [K=256]x[128,2x512] instruction = ~1.9x bf16 FLOP rate), fp32 PSUM.
  * fp8 scaling: TRN e4m3 spans [2^-9, 240]. GroupNorm output h is stored
    x8, q/k/v weights x4 (q,k,v tiles 32x true). Scores PSUM = 1024x true;
    exp applies scale SCALE/1024 and bias -2 so max e ~40 << 240 (softmax
    shift-invariance cancels the -2). The denominator matmul constant is
    32.0 so rs = 1/(32 sum e) cancels the 32x in v; the denominator is also
    evicted as fp8 at x2^-8 for the bo bias matmul (bo lhsT carries x256).

Engine assignment and emission order are tuned against the PE's in-order
queue (trace-driven): PSUM tiles are double-bank [128, 2, 512] so each
eviction/exp is one wide op; scores chunk 1 is emitted before attn@v chunk 0
so the PE has ready work while exp drains; sample s+1's GroupNorm work is
split so its tiny PE matmuls (bf16, fast path) sit in the PE queue only
where their DVE inputs are already done.
  ACT : exp, v evict (Copy), denominator fp8 evict, GroupNorm affine
        (exp is the only table function -> single table load ever)
  DVE : q/k evict+bias, bn_stats, Newton rsqrt (no ACT Sqrt), reciprocal,
        h2*rs
  Pool: final out = t + x (plain SBUF add)
  PE  : all matmuls incl. softmax denominator (32.0-matmul DoubleRow)
"""

import numpy as np
import ml_dtypes

import concourse.bacc as bacc
import concourse.bass as bass
import concourse.mybir as mybir
import concourse.tile as tile
from concourse import bass_isa
from concourse.bass_utils import run_bass_kernel_spmd

N_CORES = 8
B, C, H, W = 32, 512, 32, 32
HW = H * W                      # 1024 spatial positions
BS = B // N_CORES               # 4 samples per core
GROUPS = 32
EPS = 1e-5
SCALE = float(C) ** -0.5
P = 128
CT = C // P                     # 4 channel tiles
MT = HW // P                    # 8 spatial tiles
KP = CT // 2                    # 2 DoubleRow contraction pairs over channels
MP = MT // 2                    # 4 DoubleRow contraction pairs over spatial
NF = 512                        # matmul moving-dim chunk (output columns)
NCH = HW // NF                  # 2 column chunks
F32 = mybir.dt.float32
I32 = mybir.dt.int32
BF16 = mybir.dt.bfloat16
F8 = mybir.dt.float8e4
AX = mybir.AxisListType
ALU = mybir.AluOpType
ACTF = mybir.ActivationFunctionType
DR = mybir.MatmulPerfMode.DoubleRow

H_SC = 8.0                      # h stored as 8*h
W_SC = 4.0                      # q/k/v weights stored as 4*W
QK_SC = H_SC * W_SC             # q,k tiles are 32x true
S_SC = QK_SC * QK_SC            # scores PSUM is 1024x true
E_BIAS = -2.0                   # exp(s - 2): keeps max e ~40 << 240 (fp8 max)
ONE_V = 32.0                    # denominator matmul constant; 1/(32 sum e)
                                # cancels the 32x in the v tiles
DN_SC = 1.0 / 256.0             # denominator fp8 evict scale (keeps ~35 max)
BO_SC = 256.0                   # bo lhsT scale: bo9 @ dn8 = bo * 32 sum e
RSQRT_MAGIC = 0x5F3759DF
RSQRT_SUB = 0x7FFFFFFF - RSQRT_MAGIC


def build():
    nc = bacc.Bacc("TRN2", target_bir_lowering=False, debug=False,
                   num_devices=N_CORES, num_swdge_queues=4)

    x_d = nc.declare_dram_parameter("x", [BS, C, HW], F32, isOutput=False)
    wq_d = nc.declare_dram_parameter("wq", [P, KP, 2, C], F8, isOutput=False)
    wk_d = nc.declare_dram_parameter("wk", [P, KP, 2, C], F8, isOutput=False)
    wv_d = nc.declare_dram_parameter("wv", [P, KP, 2, C], F8, isOutput=False)
    bo9_d = nc.declare_dram_parameter("bo9", [P, C], F8, isOutput=False)
    gm_d = nc.declare_dram_parameter("gm", [P, CT, GROUPS], BF16, isOutput=False)
    gmT_d = nc.declare_dram_parameter("gmT", [GROUPS, C], BF16, isOutput=False)
    bq_d = nc.declare_dram_parameter("bq", [P, CT], F32, isOutput=False)
    bk_d = nc.declare_dram_parameter("bk", [P, CT], F32, isOutput=False)
    gnw_d = nc.declare_dram_parameter("gnw", [P, CT], F32, isOutput=False)
    gnb_d = nc.declare_dram_parameter("gnb", [P, CT], F32, isOutput=False)
    out_d = nc.declare_dram_parameter("out", [BS, C, HW], F32, isOutput=True)

    with tile.TileContext(nc) as tc:
        build_tile(tc, x_d, wq_d, wk_d, wv_d, bo9_d, gm_d, gmT_d,
                   bq_d, bk_d, gnw_d, gnb_d, out_d)
    nc.finalize()
    return nc


def build_tile(tc, x_d, wq_d, wk_d, wv_d, bo9_d, gm_d, gmT_d,
               bq_d, bk_d, gnw_d, gnb_d, out_d):
    nc = tc.nc
    from contextlib import ExitStack
    with ExitStack() as ctx:
        ctx.enter_context(nc.allow_low_precision(
            reason="fp8 DoubleRow matmuls; fp32 accumulate in PSUM"))
        consts = ctx.enter_context(tc.tile_pool(name="consts", bufs=1))
        xs = ctx.enter_context(tc.tile_pool(name="xs", bufs=12))
        hp = ctx.enter_context(tc.tile_pool(name="hp", bufs=2))
        qp = ctx.enter_context(tc.tile_pool(name="qp", bufs=2))
        kp_ = ctx.enter_context(tc.tile_pool(name="kp", bufs=2))
        vp = ctx.enter_context(tc.tile_pool(name="vp", bufs=2))
        ep = ctx.enter_context(tc.tile_pool(name="ep", bufs=4))
        rp = ctx.enter_context(tc.tile_pool(name="rp", bufs=3))
        dp = ctx.enter_context(tc.tile_pool(name="dp", bufs=3))
        op = ctx.enter_context(tc.tile_pool(name="op", bufs=10))
        st = ctx.enter_context(tc.tile_pool(name="st", bufs=24))
        pgn = ctx.enter_context(tc.tile_pool(name="pgn", bufs=1, space="PSUM"))
        pm1 = ctx.enter_context(tc.tile_pool(name="pm1", bufs=1, space="PSUM"))
        pm2 = ctx.enter_context(tc.tile_pool(name="pm2", bufs=3, space="PSUM"))

        # ---- constants / weights ----
        gm_sb = consts.tile([P, CT, GROUPS], BF16, name="gm_sb")
        gmT_sb = consts.tile([GROUPS, C], BF16, name="gmT_sb")
        bq_sb = consts.tile([P, CT], F32, name="bq_sb")
        bk_sb = consts.tile([P, CT], F32, name="bk_sb")
        gnw_sb = consts.tile([P, CT], F32, name="gnw_sb")
        gnb_sb = consts.tile([P, CT], F32, name="gnb_sb")
        epsg_sb = consts.tile([GROUPS, 1], F32, name="epsg_sb")
        ebias_sb = consts.tile([P, 1], F32, name="ebias_sb")
        ones_sb = consts.tile([P, 2, P], F8, name="ones_sb")
        bo9_sb = consts.tile([P, C], F8, name="bo9_sb")
        wq_sb = consts.tile([P, KP, 2, C], F8, name="wq_sb")
        wk_sb = consts.tile([P, KP, 2, C], F8, name="wk_sb")
        wv_sb = consts.tile([P, KP, 2, C], F8, name="wv_sb")

        for sb, d in ((gm_sb, gm_d), (gmT_sb, gmT_d), (bo9_sb, bo9_d),
                      (bq_sb, bq_d), (bk_sb, bk_d),
                      (gnw_sb, gnw_d), (gnb_sb, gnb_d)):
            nc.gpsimd.dma_start(out=sb, in_=d[:])
        nc.vector.memset(epsg_sb, EPS)
        nc.vector.memset(ebias_sb, E_BIAS)
        nc.vector.memset(ones_sb, ONE_V)

        inv_gsz = 1.0 / (C // GROUPS * HW)

        def prep_load(s):
            """x DMA for sample s."""
            x_t = []
            for ct in range(CT):
                xt = xs.tile([P, HW], F32, name=f"x_s{s}_{ct}", tag="x")
                for n in range(NCH):
                    nsl = slice(n * NF, (n + 1) * NF)
                    nc.sync.dma_start(out=xt[:, nsl],
                                      in_=x_d[s, ct * P:(ct + 1) * P, nsl])
                x_t.append(xt)
            if s == 0:
                for sb, d in ((wq_sb, wq_d), (wk_sb, wk_d), (wv_sb, wv_d)):
                    nc.gpsimd.dma_start(out=sb, in_=d[:])
            return x_t

        def stats_a_dve(s, x_t):
            """bn_stats row stats -> me tiles (bf16 [mean, E[x^2]])."""
            me_l = []
            for ct in range(CT):
                bnst = st.tile([P, NCH, 6], F32, name=f"bnst_{s}_{ct}",
                               tag=f"bnst{ct}")
                xv = x_t[ct].rearrange("p (a b) -> p a b", b=NF)
                for sg in range(NCH):
                    nc.vector.bn_stats(bnst[:, sg, :], xv[:, sg, :])
                rowmv = st.tile([P, 2], F32, name=f"rowmv_{s}_{ct}",
                                tag="rowmv")
                nc.vector.bn_aggr(rowmv, bnst)
                me = st.tile([P, 2], BF16, name=f"me_{s}_{ct}", tag="me")
                nc.vector.tensor_copy(me[:, 0:1], rowmv[:, 0:1])
                nc.vector.scalar_tensor_tensor(
                    out=me[:, 1:2], in0=rowmv[:, 0:1],
                    scalar=rowmv[:, 0:1], in1=rowmv[:, 1:2],
                    op0=ALU.mult, op1=ALU.add)
                me_l.append(me)
            return me_l

        def stats_a_pe(s, me_l):
            """group-sum matmuls -> gsum (PSUM); gm carries x1024."""
            gsum = pgn.tile([GROUPS, 2], F32, name=f"gsum_{s}", tag="pg")
            for ct in range(CT):
                nc.tensor.matmul(gsum, lhsT=gm_sb[:, ct, :], rhs=me_l[ct],
                                 start=(ct == 0), stop=(ct == CT - 1))
            return gsum

        def stats_rsqrt(s, gsum):
            """mean + Newton-rsqrt(var+eps) -> mv (bf16, for expand MM)."""
            mv = st.tile([GROUPS, 2], BF16, name=f"mv_{s}", tag="mv")
            nc.scalar.mul(out=mv[:, 0:1], in_=gsum[:, 0:1], mul=inv_gsz)
            ex2 = st.tile([GROUPS, 1], F32, name=f"ex2_{s}", tag="ex2")
            nc.scalar.mul(out=ex2, in_=gsum[:, 1:2], mul=inv_gsz)
            msq = st.tile([GROUPS, 1], F32, name=f"msq_{s}", tag="msq")
            nc.vector.tensor_mul(msq, mv[:, 0:1], mv[:, 0:1])
            vpe = st.tile([GROUPS, 1], F32, name=f"vpe_{s}", tag="vpe")
            nc.vector.tensor_sub(vpe, ex2, msq)
            nc.vector.tensor_add(vpe, vpe, epsg_sb)
            y = st.tile([GROUPS, 1], F32, name=f"y_{s}", tag="y")
            yi = y.bitcast(I32)
            vi = vpe.bitcast(I32)
            nc.vector.tensor_scalar(out=yi, in0=vi, scalar1=1,
                                    scalar2=0x7FFFFFFF,
                                    op0=ALU.logical_shift_right,
                                    op1=ALU.bitwise_xor)
            nc.vector.tensor_scalar(out=yi, in0=yi, scalar1=RSQRT_SUB,
                                    scalar2=None, op0=ALU.subtract)
            a = st.tile([GROUPS, 1], F32, name=f"a_{s}", tag="a")
            for it in range(2):
                nc.vector.tensor_mul(a, y, y)
                nc.vector.tensor_mul(a, a, vpe)
                nc.vector.tensor_scalar(out=a, in0=a, scalar1=-0.5,
                                        scalar2=1.5, op0=ALU.mult,
                                        op1=ALU.add)
                dst = mv[:, 1:2] if it == 1 else y
                nc.vector.tensor_mul(dst, y, a)
            return mv

        def stats_expand(s, mv):
            """expand (mean, rstd) to per-channel alpha/beta in one batch of
            [128, CT]-wide ops (tiny bf16 matmuls + 4 DVE ops)."""
            eps4 = pgn.tile([P, CT, 2], F32, name=f"eps4_{s}", tag="pg")
            for ct in range(CT):
                nc.tensor.matmul(eps4[:, ct, :],
                                 lhsT=gmT_sb[:, ct * P:(ct + 1) * P],
                                 rhs=mv, start=(ct == 0), stop=(ct == CT - 1),
                                 skip_group_check=True)
            exs = st.tile([P, CT, 2], F32, name=f"exs_{s}", tag="exs")
            nc.vector.tensor_copy(exs, eps4)
            alpha = st.tile([P, CT], F32, name=f"al_{s}", tag="al")
            nc.vector.tensor_mul(alpha, gnw_sb, exs[:, :, 1])
            mal = st.tile([P, CT], F32, name=f"mal_{s}", tag="mal")
            nc.vector.tensor_mul(mal, exs[:, :, 0], alpha)
            beta = st.tile([P, CT], F32, name=f"be_{s}", tag="be")
            nc.vector.tensor_sub(beta, gnb_sb, mal)
            return alpha, beta

        def stats_affine(s, x_t, alpha, beta):
            """ACT affine -> h (fp8, 8x scaled)."""
            ht = hp.tile([P, CT, HW], F8, name=f"h_{s}", tag="h")
            for ct in range(CT):
                nc.scalar.activation(out=ht[:, ct, :], in_=x_t[ct],
                                     func=ACTF.Identity,
                                     bias=beta[:, ct:ct + 1],
                                     scale=alpha[:, ct:ct + 1])
            return ht

        def body_qkv(s, h_t):
            """q, k (channel-major) and vT (spatial-major) for sample s."""
            q_t = qp.tile([P, CT, HW], F8, name=f"q_{s}", tag="q")
            k_t = kp_.tile([P, CT, HW], F8, name=f"k_{s}", tag="k")
            for w_sb, b_sb, dst, tag in ((wq_sb, bq_sb, q_t, "q"),
                                         (wk_sb, bk_sb, k_t, "k")):
                for mt in range(CT):
                    ps2 = pm2.tile([P, NCH, NF], F32,
                                   name=f"{tag}p_{s}_{mt}", tag="ps")
                    for kpi in range(KP):
                        lhsT = w_sb[:, kpi, :, mt * P:(mt + 1) * P]
                        for n in range(NCH):
                            nsl = slice(n * NF, (n + 1) * NF)
                            nc.tensor.matmul(
                                ps2[:, n, :], lhsT=lhsT,
                                rhs=h_t[:, 2 * kpi:2 * kpi + 2, nsl],
                                start=(kpi == 0), stop=(kpi == KP - 1),
                                perf_mode=DR)
                    nc.vector.tensor_scalar(
                        out=dst[:, mt, :],
                        in0=ps2.rearrange("p a b -> p (a b)"),
                        scalar1=b_sb[:, mt:mt + 1], scalar2=None,
                        op0=ALU.add)

            vT = vp.tile([P, MT, C], F8, name=f"v_{s}", tag="v")
            for mtp in range(MT // 2):
                ps2 = pm2.tile([P, 2, C], F32, name=f"vp_{s}_{mtp}", tag="ps")
                for i in range(2):
                    mt = 2 * mtp + i
                    for kpi in range(KP):
                        nc.tensor.matmul(
                            ps2[:, i, :],
                            lhsT=h_t[:, 2 * kpi:2 * kpi + 2,
                                     mt * P:(mt + 1) * P],
                            rhs=wv_sb[:, kpi, :, :],
                            start=(kpi == 0), stop=(kpi == KP - 1),
                            perf_mode=DR)
                nc.scalar.activation(out=vT[:, 2 * mtp:2 * mtp + 2, :],
                                     in_=ps2, func=ACTF.Copy)
            return q_t, k_t, vT

        def attn_scores(s, n, q_t, k_t):
            """scores + exp for column chunk n -> e super-tile."""
            nsl = slice(n * NF, (n + 1) * NF)
            e_t = ep.tile([P, MT, NF], F8, name=f"e_{s}_{n}", tag="e")
            for mtp in range(MT // 2):
                ps2 = pm2.tile([P, 2, NF], F32, name=f"ep_{s}_{n}_{mtp}",
                               tag="ps")
                for i in range(2):
                    mt = 2 * mtp + i
                    for kpi in range(KP):
                        nc.tensor.matmul(
                            ps2[:, i, :],
                            lhsT=k_t[:, 2 * kpi:2 * kpi + 2,
                                     mt * P:(mt + 1) * P],
                            rhs=q_t[:, 2 * kpi:2 * kpi + 2, nsl],
                            start=(kpi == 0), stop=(kpi == KP - 1),
                            perf_mode=DR)
                nc.scalar.activation(out=e_t[:, 2 * mtp:2 * mtp + 2, :],
                                     in_=ps2, func=ACTF.Exp,
                                     scale=SCALE / S_SC, bias=ebias_sb)
            return e_t

        def attn_out(s, n, x_t, vT, e_t, last=False):
            """denominator + (v' @ e^T ... + bo-bias matmul) * rs + x,
            interleaved as mp-waves so the PE starts on the first exp
            pairs immediately."""
            nsl = slice(n * NF, (n + 1) * NF)
            den = pm1.tile([P, NF], F32, name=f"dn_{s}_{n}", tag="pd")
            h2 = [pm2.tile([P, 2, NF], F32, name=f"h2_{s}_{n}_{cp}", tag="ps")
                  for cp in range(CT // 2)]
            for mp in range(MP):
                epair = e_t[:, 2 * mp:2 * mp + 2, :]
                nc.tensor.matmul(den, lhsT=ones_sb, rhs=epair,
                                 start=(mp == 0), stop=(mp == MP - 1),
                                 perf_mode=DR)
                for cp in range(CT // 2):
                    for i in range(2):
                        ct = 2 * cp + i
                        nc.tensor.matmul(
                            h2[cp][:, i, :],
                            lhsT=vT[:, 2 * mp:2 * mp + 2,
                                    ct * P:(ct + 1) * P],
                            rhs=epair,
                            start=(mp == 0), stop=False,
                            perf_mode=DR)
            # denominator: fp8 evict (for the bo bias matmul) + reciprocal
            dn8 = dp.tile([P, NF], F8, name=f"dn8_{s}_{n}", tag="dn8")
            nc.vector.tensor_scalar_mul(dn8, den, DN_SC)
            rs = rp.tile([P, NF], F32, name=f"rs_{s}_{n}", tag="rs")
            nc.vector.reciprocal_approx_fast(out=rs, in_=den)
            # close each h2 group with the rank-1 bo matmul:
            # h2 += bo9[:, ct-slice].T @ dn8 = 256*bo * (32 sum e)/256
            for cp in range(CT // 2):
                for i in range(2):
                    ct = 2 * cp + i
                    nc.tensor.matmul(
                        h2[cp][:, i, :],
                        lhsT=bo9_sb[:, ct * P:(ct + 1) * P],
                        rhs=dn8, start=False, stop=True,
                        skip_group_check=True)
            for cp in range(CT // 2):
                for i in range(2):
                    ct = 2 * cp + i
                    t_sb = op.tile([P, NF], F32, name=f"t_{s}_{n}_{ct}",
                                   tag="t")
                    nc.vector.tensor_mul(t_sb, h2[cp][:, i, :], rs)
                    o_sb = op.tile([P, NF], F32, name=f"o_{s}_{n}_{ct}",
                                   tag="o")
                    # last chunk: split the serial drain across DVE and Pool
                    if last and ct >= 2:
                        nc.vector.tensor_add(o_sb, t_sb, x_t[ct][:, nsl])
                    else:
                        nc.gpsimd.tensor_add(o_sb, t_sb, x_t[ct][:, nsl])
                    nc.sync.dma_start(
                        out=out_d[s, ct * P:(ct + 1) * P, nsl], in_=o_sb)

        # ---- software pipeline over samples, GroupNorm 2 samples deep:
        # iteration s runs bn/rsqrt for s+2 in DVE slack and expand/affine
        # for s+1 right after body_qkv(s), so h(s) is always ready before
        # qkv(s) with no PE stall on the stats chain ----
        xm = {0: prep_load(0), 1: prep_load(1)}
        hm = {}
        me0 = stats_a_dve(0, xm[0])
        g0 = stats_a_pe(0, me0)
        mv0 = stats_rsqrt(0, g0)
        a0, b0 = stats_expand(0, mv0)
        hm[0] = stats_affine(0, xm[0], a0, b0)
        me1 = stats_a_dve(1, xm[1])
        g1 = stats_a_pe(1, me1)
        pending = (1, stats_rsqrt(1, g1))
        for s in range(BS):
            q_t, k_t, vT = body_qkv(s, hm[s])
            if pending is not None:
                sp, mvp = pending
                pending = None
                abp = stats_expand(sp, mvp)
                hm[sp] = stats_affine(sp, xm[sp], *abp)
            e0 = attn_scores(s, 0, q_t, k_t)
            # scores chunk 1 before attn chunk 0: the PE has ready work
            # while ACT drains exp of chunk 0
            e1 = attn_scores(s, 1, q_t, k_t)
            if s + 2 < BS:
                xm[s + 2] = prep_load(s + 2)
            attn_out(s, 0, xm[s], vT, e0)
            me_n = stats_a_dve(s + 2, xm[s + 2]) if s + 2 < BS else None
            attn_out(s, 1, xm[s], vT, e1, last=(s == BS - 1))
            if me_n is not None:
                gs = stats_a_pe(s + 2, me_n)
                pending = (s + 2, stats_rsqrt(s + 2, gs))


_NC_CACHE = None


def _get_nc():
    global _NC_CACHE
    if _NC_CACHE is None:
        _NC_CACHE = build()
    return _NC_CACHE


F8NP = ml_dtypes.float8_e4m3


def _tile_w_dr(w):
    """[512 out, 512 in] weight -> DoubleRow lhsT tiles [P, KP, 2, C]:
    [p, kp, i, o] = w[o, (kp*2 + i)*128 + p], scaled by W_SC, fp8."""
    wT = (W_SC * w.T).astype(np.float32)          # [c_in, o]
    return np.ascontiguousarray(
        wT.reshape(KP, 2, P, C).transpose(2, 0, 1, 3)).astype(F8NP)


def _tile_vec(v, scale=1.0):
    """[512] -> [128, 4] per-partition scalars: [p, kt] = scale*v[kt*128+p]"""
    return np.ascontiguousarray(
        (scale * np.asarray(v, dtype=np.float64)).astype(np.float32)
        .reshape(CT, P).T)


def make_in_maps(x, gn_w, gn_b, qkv_w, qkv_b, proj_w, proj_b):
    x = np.asarray(x, dtype=np.float32)
    gn_w = np.asarray(gn_w, dtype=np.float32)
    gn_b = np.asarray(gn_b, dtype=np.float32)
    qkv_w = np.asarray(qkv_w, dtype=np.float32)
    qkv_b = np.asarray(qkv_b, dtype=np.float32)
    proj_w = np.asarray(proj_w, dtype=np.float32)
    proj_b = np.asarray(proj_b, dtype=np.float32)

    xr = x.reshape(B, C, HW)
    gmat = np.kron(np.eye(GROUPS, dtype=np.float32),
                   np.ones((C // GROUPS, 1), dtype=np.float32))  # [512, 32]
    gm_t = (float(HW) * gm_t_base(gmat)).astype(ml_dtypes.bfloat16)
    gmT_t = np.ascontiguousarray(gmat.T).astype(ml_dtypes.bfloat16)

    # fold proj into v: W' = proj_w @ wv; bias collapses to a constant
    # output offset bo = proj_w @ bv + pb (softmax rows sum to 1)
    wv_folded = proj_w @ qkv_w[2 * C:3 * C]
    bo = proj_w @ qkv_b[2 * C:3 * C] + proj_b
    bo9 = np.zeros((P, C), dtype=np.float32)
    bo9[0, :] = BO_SC * bo

    common = {
        "wq": _tile_w_dr(qkv_w[0:C]),
        "wk": _tile_w_dr(qkv_w[C:2 * C]),
        "wv": _tile_w_dr(wv_folded),
        "bo9": bo9.astype(F8NP),
        "gm": gm_t,
        "gmT": gmT_t,
        "bq": _tile_vec(qkv_b[0:C], QK_SC),
        "bk": _tile_vec(qkv_b[C:2 * C], QK_SC),
        "gnw": _tile_vec(gn_w, H_SC),
        "gnb": _tile_vec(gn_b, H_SC),
    }
    in_maps = []
    for c in range(N_CORES):
        m = dict(common)
        m["x"] = np.ascontiguousarray(xr[c * BS:(c + 1) * BS])
        in_maps.append(m)
    return in_maps


def gm_t_base(gmat):
    return np.ascontiguousarray(
        gmat.reshape(CT, P, GROUPS).transpose(1, 0, 2)).astype(np.float32)


def kernel(**inputs):
    in_maps = make_in_maps(**inputs)
    nc = _get_nc()
    res = run_bass_kernel_spmd(nc, in_maps, core_ids=list(range(N_CORES)))
    out = np.concatenate([res.results[c]["out"] for c in range(N_CORES)],
                         axis=0)
    return out.reshape(B, C, H, W).astype(np.float32)
